# revision 54
# baseline (speedup 1.0000x reference)
"""Trainium2 Bass kernel for nn_DLI_loss_full.

Key algebraic fact: logits[b,j,k] = hw[b,j] + xw[b,k] and the loss is
sum(lse - tgt) over valid groups, so the hw[b,j] term (the whole LSTM
path) cancels exactly:

    per_group[b,j] = log(sum_{k=j+1}^{len_b-1} exp(xw[b,k])) - xw[b,j+1]
    loss = sum(per_group) / sum_b(len_b - 1)

with xw = encoder_output @ w_fc[HID:].  The kernel streams
encoder_output once (memory-bound, ~351 B/ns with all 16 DMA engines
saturated) on the sync HWDGE queue — the sync sequencer has nothing
else to do, so descriptor generation never delays compute issue — and
computes the per-timestep dot products as a two-engine pipeline with
no cross-engine back-edges (so the compile-time schedule cannot
head-of-line-block an engine):

  - scalar (ACT) casts each landed fp32 piece to bf16, and one piece's
    256-wide reductions run there via Copy+accum_out in the slack
    between casts,
  - vector (DVE) multiplies in the 2x 16-bit perf mode and reduces via
    two bf16 halving adds (2x mode) plus a short 1x reduce.

Pieces are sized 3+6*7+3 rows: a small first piece starts compute
~2.5us earlier and a small last piece keeps the post-stream tail
short.  tile_wait_until hints align the scheduler's per-engine
instruction order with the real DMA landing cadence.
Small constants (masks, um) ride the gpsimd SWDGE queue; they are only
needed in the tail.  A manual LoadActFuncSet of
natural_log_exp_and_others at the top of the scalar stream serves Exp
and Ln with a single hidden table load.  The valid-k mask folds into
xw as an additive -60000 bias so a single Exp+accum_out produces both
the masked exponentials and the chunk totals; suffix log-sum-exps come
from one per-chunk scan seeded through a 128x128 bf16 matmul.
"""

from contextlib import ExitStack

import ml_dtypes
import numpy as np

import concourse.bacc as bacc
import concourse.mybir as mybir
import concourse.tile as tile
from concourse import bass_utils

B, T, D, HID = 128, 384, 256, 256
NCORES = 8
BS = B // NCORES            # 16 batches per core
CH = 8                      # chunks per sequence
L = T // CH                 # 48 timesteps per chunk
P = BS * CH                 # 128 partitions
NP = 8                      # DMA/compute pieces along the free axis
LP = L // NP                # 6 timesteps per piece
F32 = mybir.dt.float32
BF16 = mybir.dt.bfloat16
EPS = 1e-30                 # keeps ln() finite on fully-masked tails
MASK_NEG = -73.0            # exp() ~ 2e-32: negligible yet nonzero, so
                            # suffix sums never hit ln(0) and no EPS
                            # seed is needed
ATL_LN_EXP = 6              # act_info.json index of natural_log_exp_and_others

_cache = {}


def _build_nc():
    nc = bacc.Bacc(
        "TRN2", target_bir_lowering=False, debug=False, num_devices=NCORES
    )
    x = nc.dram_tensor("x", [BS, T, D], F32, kind="ExternalInput").ap()
    wt = nc.dram_tensor("wt", [P, D], F32, kind="ExternalInput").ap()
    cb = nc.dram_tensor("cb", [P, 3 * L], F32, kind="ExternalInput").ap()
    c2 = nc.dram_tensor("c2", [P, P], BF16, kind="ExternalInput").ap()
    out = nc.dram_tensor("out", [P, 3], F32, kind="ExternalOutput").ap()

    add = mybir.AluOpType.add
    mult = mybir.AluOpType.mult
    bypass = mybir.AluOpType.bypass
    AX = mybir.AxisListType.X
    ACT = mybir.ActivationFunctionType

    with tile.TileContext(nc) as tc, ExitStack() as ctx, \
            nc.allow_low_precision(reason="bf16 2x-mode dot products; loss tolerance is 2e-2"):
        sp = ctx.enter_context(tc.tile_pool(name="small", bufs=1))
        xp = ctx.enter_context(tc.tile_pool(name="xp", bufs=10))
        bp = ctx.enter_context(tc.tile_pool(name="bp", bufs=4))
        b2 = ctx.enter_context(tc.tile_pool(name="b2", bufs=2))
        pp = ctx.enter_context(tc.tile_pool(name="psum", bufs=1, space="PSUM"))

        # piece row extents: small first piece so compute starts early,
        # small last piece so the post-stream tail is short
        ROWS = (3, 6, 6, 6, 6, 6, 6, 6, 3)
        R0 = [sum(ROWS[:i]) for i in range(len(ROWS))]
        NPC = len(ROWS)
        ACT_RED = 1         # this piece's reductions run on scalar Copy+accum
        GP_A = GP_V = -1    # gpsimd tail offload: measured slower, disabled

        # expected piece-i landing time in the scheduler's simulated
        # clock (serial DMA queue model): used only to pin instruction
        # order per engine, no hardware waits are emitted
        def tw(us):
            return tc.tile_wait_until(us * 1e-3)

        t_piece = []
        t = 0.8
        for r in ROWS:
            t += 2.37 * r / 6.0
            t_piece.append(t)

        # one table load serves every Exp/Ln below; runs at stream head
        # while w / the first x piece are still in flight
        nc.scalar.add_instruction(
            mybir.InstLoadActFuncSet(
                name="manual_atl", act_func_set_id=ATL_LN_EXP, ins=[], outs=[]
            )
        )

        # w first, then the x pieces, all on the sync HWDGE queue: 128
        # descriptors of 6KB per piece keep all 16 DMA engines saturated,
        # and the sync sequencer has nothing else to do, so descriptor
        # generation never blocks compute issue (on the scalar queue the
        # piece descriptor-gens delayed every activation by ~7us)
        w_sb = sp.tile([P, D], F32)
        nc.sync.dma_start(w_sb[:], wt)
        x_p = x.rearrange("b (c l) d -> (b c) (l d)", c=CH)
        xts = []
        for i in range(NPC):
            xt = xp.tile([P, LP * D], F32, tag="x")
            nc.sync.dma_start(
                xt[:, 0:ROWS[i] * D],
                x_p[:, R0[i] * D:(R0[i] + ROWS[i]) * D],
            )
            xts.append(xt)

        # small tail-only constants ride the gpsimd SWDGE queue
        c_sb = sp.tile([P, 3 * L], F32)
        nc.gpsimd.dma_start(c_sb[:], cb)
        mf_sb = c_sb[:, 0:L]
        wm_sb = c_sb[:, L:2 * L]
        mb_sb = c_sb[:, 2 * L:3 * L]
        umb = sp.tile([P, P], BF16)
        nc.gpsimd.dma_start(umb[:], c2)

        # bf16 weight replica, built while the first piece is in flight
        wtb = sp.tile([P, D], BF16)
        nc.scalar.activation(wtb[:], w_sb[:], ACT.Copy)
        wb = sp.tile([P, LP * D], BF16)
        nc.vector.tensor_copy(wb[:, 0:D], wtb[:])
        nc.vector.tensor_copy(wb[:, D:2 * D], wb[:, 0:D])
        nc.vector.tensor_copy(wb[:, 2 * D:4 * D], wb[:, 0:2 * D])
        nc.vector.tensor_copy(wb[:, 4 * D:6 * D], wb[:, 2 * D:4 * D])
        w3b = wb[:].rearrange("p (l d) -> p l d", d=D)

        res = sp.tile([P, 3], F32)
        # valid-group count is independent of x: do it up front
        nc.vector.tensor_reduce(res[:, 1:2], mf_sb, axis=AX, op=add)

        # xw[p, t] = sum_d x[p, t, d] * w[d]
        xw = sp.tile([P, L], F32)
        ascr = sp.tile([P, D], BF16)

        # fp32 weight replica (3 rows) for gpsimd's last-piece multiply
        wf3 = sp.tile([P, 3 * D], F32)
        ascr2 = sp.tile([P, D], F32)
        nc.gpsimd.tensor_copy(wf3[:, 0:D], w_sb[:])
        nc.gpsimd.tensor_copy(wf3[:, D:2 * D], wf3[:, 0:D])
        nc.gpsimd.tensor_copy(wf3[:, 2 * D:3 * D], wf3[:, 0:D])

        act_red = []        # deferred (l, xb) scalar-engine reductions
        xb_hold = sp.tile([P, LP * D], BF16)   # ACT_RED piece: no pool reuse
        pair_buf = None
        for i in range(NPC):
            rows = ROWS[i]
            if i == NPC - 1:
                # last piece entirely off the vector engine (the gap-free
                # critical path): gpsimd multiplies the landed fp32 tile
                # in place at arrival, scalar reduces the product
                x3g = xts[i][:, 0:rows * D].rearrange("p (l d) -> p l d", d=D)
                w3g = wf3[:, 0:rows * D].rearrange("p (l d) -> p l d", d=D)
                with tw(t_piece[i]):
                    nc.gpsimd.tensor_tensor(x3g, x3g, w3g, mult)
                for l in range(rows):
                    t = R0[i] + l
                    with tw(t_piece[i] + 0.8 + 0.3 * l):
                        nc.scalar.activation(
                            ascr2[:], xts[i][:, l * D:(l + 1) * D],
                            ACT.Copy, accum_out=xw[:, t:t + 1],
                        )
                continue
            # pieces 2..5 group as a quad and 6..7 as a pair in shared
            # bf16 tiles so the reduction stages run once per group
            # (per-instruction overhead is ~25% of the stage cost)
            in_pair = 2 <= i <= 7
            gsz = 4 if i <= 5 else 2
            g0 = 2 if i <= 5 else 6
            if in_pair and i == g0:
                pair_buf = b2.tile(
                    [P, gsz * LP * D], BF16, tag=f"g{g0}", name="xbg"
                )
            if i == ACT_RED:
                xb, off = xb_hold, 0
            elif in_pair:
                xb, off = pair_buf, (i - g0) * LP * D
            else:
                xb = bp.tile([P, LP * D], BF16, tag="xb", name="xbs")
                off = 0
            x3 = xb[:, off:off + rows * D].rearrange("p (l d) -> p l d", d=D)
            w3 = wb[:, 0:rows * D].rearrange("p (l d) -> p l d", d=D)
            if i in (1, 2, 3):
                # vector casts these itself: it is otherwise idle waiting
                # for the scalar cast chain here (arrival-gated gaps), and
                # this relieves the scalar engine for the later casts
                with tw(t_piece[i] + 0.1):
                    nc.vector.tensor_copy(
                        xb[:, off:off + rows * D], xts[i][:, 0:rows * D]
                    )
            else:
                with tw(t_piece[i]):
                    nc.scalar.activation(
                        xb[:, off:off + rows * D], xts[i][:, 0:rows * D],
                        ACT.Copy,
                    )
            with tw(t_piece[i] + 0.25 * rows):
                nc.vector.tensor_tensor(x3, x3, w3, mult)
            if i == ACT_RED:
                act_red = [(R0[i] + l, xb) for l in range(rows)]
                continue
            if in_pair and i == g0 + gsz - 1:
                # staged reduce over the whole group at once
                p3 = pair_buf[:].rearrange("p (l d) -> p l d", d=D)
                with tw(t_piece[i] + 0.25 * rows + 0.9):
                    nc.vector.tensor_tensor(
                        p3[:, :, 0:128], p3[:, :, 0:128], p3[:, :, 128:256], add
                    )
                    nc.vector.tensor_tensor(
                        p3[:, :, 0:64], p3[:, :, 0:64], p3[:, :, 64:128], add
                    )
                    nc.vector.tensor_reduce(
                        xw[:, R0[g0]:R0[g0] + gsz * LP], p3[:, :, 0:64],
                        axis=AX, op=add,
                    )
            elif i == NPC - 1:
                # last piece's reductions run on the scalar engine, which
                # is idle by now — vector is the gap-free critical path
                # from ~19us to the end
                for l in range(rows):
                    t = R0[i] + l
                    with tw(t_piece[i] + 0.7 + 0.3 * l):
                        nc.scalar.activation(
                            ascr[:], xb[:, off + l * D:off + (l + 1) * D],
                            ACT.Copy, accum_out=xw[:, t:t + 1],
                        )
            elif not in_pair:
                with tw(t_piece[i] + 0.25 * rows + 0.9):
                    nc.vector.tensor_reduce(
                        xw[:, R0[i]:R0[i] + rows], x3, axis=AX, op=add
                    )
            # one deferred scalar-engine reduction per later piece: fills
            # the scalar engine's slack between casts
            if act_red and i >= 2:
                t, axb = act_red.pop(0)
                with tw(t_piece[i] + 1.8):
                    nc.scalar.activation(
                        ascr[:], axb[:, (t - R0[ACT_RED]) * D:
                                      (t - R0[ACT_RED] + 1) * D],
                        ACT.Copy, accum_out=xw[:, t:t + 1],
                    )

        # any leftovers (fewer later pieces than ACT_RED rows)
        for t, axb in act_red:
            with tw(t_piece[-1] + 1.0):
                nc.scalar.activation(
                    ascr[:], axb[:, (t - R0[ACT_RED]) * D:
                                  (t - R0[ACT_RED] + 1) * D],
                    ACT.Copy, accum_out=xw[:, t:t + 1],
                )

        TT = t_piece[-1] + 1.6
        # fold the valid-k mask into xw (masked -> -73, exp -> ~2e-32:
        # negligible in every suffix sum yet keeps ln() finite, so no
        # EPS seed is needed); wm is 0 there so loss terms are unaffected
        with tw(TT):
            nc.vector.tensor_add(xw[:], xw[:], mb_sb)
        # masked exponentials and chunk totals in one activation
        em = sp.tile([P, L], F32)
        tot = sp.tile([P, 1], BF16)
        with tw(TT + 0.1):
            nc.scalar.activation(em[:], xw[:], ACT.Exp, accum_out=tot[:])
        # sum(wm * xw) runs on vector in parallel with the exp: only the
        # ln-dependent half of the loss sum stays on the critical path
        diff = sp.tile([P, L], F32)
        with tw(TT + 0.1):
            nc.vector.scalar_tensor_tensor(
                out=diff[:], in0=xw[:], scalar=1.0, in1=wm_sb,
                op0=bypass, op1=mult, accum_out=res[:, 2:3],
            )
        # cross-chunk exclusive suffix totals via 128x128 bf16 matmul
        aps = pp.tile([P, 1], F32, tag="mm")
        with tw(TT + 0.2):
            nc.tensor.matmul(aps[:], umb[:], tot[:], start=True, stop=True)

        # within-chunk suffix sums run concurrently with the matmul
        # (seed 0); the later-chunk total is broadcast-added from PSUM
        ss = sp.tile([P, L], F32)
        with tw(TT + 0.25):
            nc.vector.tensor_tensor_scan(
                ss[:][:, ::-1], em[:][:, ::-1], em[:][:, ::-1],
                initial=0.0, op0=add, op1=bypass,
            )
        with tw(TT + 0.4):
            nc.vector.tensor_scalar_add(ss[:], ss[:], aps[:])
        lt = sp.tile([P, L], F32)
        with tw(TT + 0.5):
            nc.scalar.activation(lt[:], ss[:], ACT.Ln)

        # sum(wm * ln(suffix)); host computes num = col0 - col2
        with tw(TT + 0.6):
            nc.vector.scalar_tensor_tensor(
                out=diff[:], in0=lt[:], scalar=1.0, in1=wm_sb,
                op0=bypass, op1=mult, accum_out=res[:, 0:1],
            )
        with tw(TT + 0.8):
            nc.scalar.dma_start(out, res[:])

    nc.compile()
    return nc


def _host_consts():
    w_idx = np.arange(P)
    um = (
        (w_idx[:, None] // CH == w_idx[None, :] // CH)
        & (w_idx[:, None] % CH > w_idx[None, :] % CH)
    ).astype(np.float32)
    cm = np.ones((P, L), np.float32)
    cm[w_idx % CH == 0, 0] = 0.0
    return um, cm


def _host_blobs(mask, w_fc):
    """wt fp32, per-core mask blobs fp32, um bf16."""
    um, cm = _host_consts()
    wt = np.ascontiguousarray(
        np.broadcast_to(w_fc[HID:], (P, D)), np.float32
    )
    c2 = np.ascontiguousarray(um.astype(ml_dtypes.bfloat16))
    mfs = mask.astype(np.float32).reshape(NCORES, P, L)
    cbs = []
    for c in range(NCORES):
        mf = mfs[c]
        wm = mf * cm
        mb = (1.0 - mf) * MASK_NEG
        cbs.append(np.ascontiguousarray(
            np.concatenate([mf, wm, mb], axis=1), np.float32
        ))
    return wt, cbs, c2


def kernel(**inputs) -> np.ndarray:
    enc = np.ascontiguousarray(np.asarray(inputs["encoder_output"], np.float32))
    mask = np.ascontiguousarray(np.asarray(inputs["mask"], np.int32))
    w_fc = np.asarray(inputs["w_fc"], np.float32)

    if "nc" not in _cache:
        _cache["nc"] = _build_nc()
    nc = _cache["nc"]

    wt, cbs, c2 = _host_blobs(mask, w_fc)
    in_maps = [
        {"x": enc[c * BS:(c + 1) * BS], "wt": wt, "cb": cbs[c], "c2": c2}
        for c in range(NCORES)
    ]
    res = bass_utils.run_bass_kernel_spmd(
        nc, in_maps, core_ids=list(range(NCORES))
    )
    o = np.stack([r["out"] for r in res.results]).astype(np.float64)
    num = o[:, :, 0].sum() - o[:, :, 2].sum()
    den = o[:, :, 1].sum() - B
    return np.asarray(num / den, dtype=np.float32)


# revision 55
# speedup vs baseline: 1.0272x; 1.0272x over previous
"""Trainium2 Bass kernel for nn_DLI_loss_full.

Key algebraic fact: logits[b,j,k] = hw[b,j] + xw[b,k] and the loss is
sum(lse - tgt) over valid groups, so the hw[b,j] term (the whole LSTM
path) cancels exactly:

    per_group[b,j] = log(sum_{k=j+1}^{len_b-1} exp(xw[b,k])) - xw[b,j+1]
    loss = sum(per_group) / sum_b(len_b - 1)

with xw = encoder_output @ w_fc[HID:].  The kernel streams
encoder_output once (memory-bound, ~351 B/ns with all 16 DMA engines
saturated) on the sync HWDGE queue — the sync sequencer has nothing
else to do, so descriptor generation never delays compute issue — and
computes the per-timestep dot products as a two-engine pipeline with
no cross-engine back-edges (so the compile-time schedule cannot
head-of-line-block an engine):

  - scalar (ACT) casts each landed fp32 piece to bf16, and one piece's
    256-wide reductions run there via Copy+accum_out in the slack
    between casts,
  - vector (DVE) multiplies in the 2x 16-bit perf mode and reduces via
    two bf16 halving adds (2x mode) plus a short 1x reduce.

Pieces are sized 3+6*7+3 rows: a small first piece starts compute
~2.5us earlier and a small last piece keeps the post-stream tail
short.  tile_wait_until hints align the scheduler's per-engine
instruction order with the real DMA landing cadence.
Small constants (masks, um) ride the gpsimd SWDGE queue; they are only
needed in the tail.  A manual LoadActFuncSet of
natural_log_exp_and_others at the top of the scalar stream serves Exp
and Ln with a single hidden table load.  The valid-k mask folds into
xw as an additive -60000 bias so a single Exp+accum_out produces both
the masked exponentials and the chunk totals; suffix log-sum-exps come
from one per-chunk scan seeded through a 128x128 bf16 matmul.
"""

from contextlib import ExitStack

import ml_dtypes
import numpy as np

import concourse.bacc as bacc
import concourse.mybir as mybir
import concourse.tile as tile
from concourse import bass_utils

B, T, D, HID = 128, 384, 256, 256
NCORES = 8
BS = B // NCORES            # 16 batches per core
CH = 8                      # chunks per sequence
L = T // CH                 # 48 timesteps per chunk
P = BS * CH                 # 128 partitions
NP = 8                      # DMA/compute pieces along the free axis
LP = L // NP                # 6 timesteps per piece
F32 = mybir.dt.float32
BF16 = mybir.dt.bfloat16
EPS = 1e-30                 # keeps ln() finite on fully-masked tails
MASK_NEG = -73.0            # exp() ~ 2e-32: negligible yet nonzero, so
                            # suffix sums never hit ln(0) and no EPS
                            # seed is needed
ATL_LN_EXP = 6              # act_info.json index of natural_log_exp_and_others

_cache = {}


def _build_nc():
    nc = bacc.Bacc(
        "TRN2", target_bir_lowering=False, debug=False, num_devices=NCORES
    )
    x = nc.dram_tensor("x", [BS, T, D], F32, kind="ExternalInput").ap()
    wt = nc.dram_tensor("wt", [P, D], F32, kind="ExternalInput").ap()
    cb = nc.dram_tensor("cb", [P, 3 * L], F32, kind="ExternalInput").ap()
    c2 = nc.dram_tensor("c2", [P, P], BF16, kind="ExternalInput").ap()
    out = nc.dram_tensor("out", [P, 3], F32, kind="ExternalOutput").ap()

    add = mybir.AluOpType.add
    mult = mybir.AluOpType.mult
    bypass = mybir.AluOpType.bypass
    AX = mybir.AxisListType.X
    ACT = mybir.ActivationFunctionType

    with tile.TileContext(nc) as tc, ExitStack() as ctx, \
            nc.allow_low_precision(reason="bf16 2x-mode dot products; loss tolerance is 2e-2"):
        sp = ctx.enter_context(tc.tile_pool(name="small", bufs=1))
        xp = ctx.enter_context(tc.tile_pool(name="xp", bufs=10))
        bp = ctx.enter_context(tc.tile_pool(name="bp", bufs=4))
        b2 = ctx.enter_context(tc.tile_pool(name="b2", bufs=2))
        pp = ctx.enter_context(tc.tile_pool(name="psum", bufs=1, space="PSUM"))

        # piece row extents: small first piece so compute starts early,
        # small last piece so the post-stream tail is short
        ROWS = (3, 6, 6, 6, 6, 6, 6, 6, 3)
        R0 = [sum(ROWS[:i]) for i in range(len(ROWS))]
        NPC = len(ROWS)
        ACT_RED = 1         # this piece's reductions run on scalar Copy+accum
        GP_A = GP_V = -1    # gpsimd tail offload: measured slower, disabled

        # expected piece-i landing time in the scheduler's simulated
        # clock (serial DMA queue model): used only to pin instruction
        # order per engine, no hardware waits are emitted
        def tw(us):
            return tc.tile_wait_until(us * 1e-3)

        t_piece = []
        t = 0.8
        for r in ROWS:
            t += 2.37 * r / 6.0
            t_piece.append(t)

        # one table load serves every Exp/Ln below; runs at stream head
        # while w / the first x piece are still in flight
        nc.scalar.add_instruction(
            mybir.InstLoadActFuncSet(
                name="manual_atl", act_func_set_id=ATL_LN_EXP, ins=[], outs=[]
            )
        )

        # w first, then the x pieces, all on the sync HWDGE queue: 128
        # descriptors of 6KB per piece keep all 16 DMA engines saturated,
        # and the sync sequencer has nothing else to do, so descriptor
        # generation never blocks compute issue (on the scalar queue the
        # piece descriptor-gens delayed every activation by ~7us)
        w_sb = sp.tile([P, D], F32)
        nc.sync.dma_start(w_sb[:], wt)
        x_p = x.rearrange("b (c l) d -> (b c) (l d)", c=CH)
        xts = []
        for i in range(NPC):
            xt = xp.tile([P, LP * D], F32, tag="x")
            nc.sync.dma_start(
                xt[:, 0:ROWS[i] * D],
                x_p[:, R0[i] * D:(R0[i] + ROWS[i]) * D],
            )
            xts.append(xt)

        # small tail-only constants ride the gpsimd SWDGE queue
        c_sb = sp.tile([P, 3 * L], F32)
        nc.gpsimd.dma_start(c_sb[:], cb)
        mf_sb = c_sb[:, 0:L]
        wm_sb = c_sb[:, L:2 * L]
        mb_sb = c_sb[:, 2 * L:3 * L]
        umb = sp.tile([P, P], BF16)
        nc.gpsimd.dma_start(umb[:], c2)

        # bf16 weight replica, built while the first piece is in flight
        wtb = sp.tile([P, D], BF16)
        nc.scalar.activation(wtb[:], w_sb[:], ACT.Copy)
        wb = sp.tile([P, LP * D], BF16)
        nc.vector.tensor_copy(wb[:, 0:D], wtb[:])
        nc.vector.tensor_copy(wb[:, D:2 * D], wb[:, 0:D])
        nc.vector.tensor_copy(wb[:, 2 * D:4 * D], wb[:, 0:2 * D])
        nc.vector.tensor_copy(wb[:, 4 * D:6 * D], wb[:, 2 * D:4 * D])
        w3b = wb[:].rearrange("p (l d) -> p l d", d=D)

        res = sp.tile([P, 3], F32)
        # valid-group count is independent of x: do it up front
        nc.vector.tensor_reduce(res[:, 1:2], mf_sb, axis=AX, op=add)

        # xw[p, t] = sum_d x[p, t, d] * w[d]
        xw = sp.tile([P, L], F32)
        ascr = sp.tile([P, D], BF16)

        # fp32 weight replica (3 rows) for gpsimd's last-piece multiply
        wf3 = sp.tile([P, 3 * D], F32)
        ascr2 = sp.tile([P, D], F32)
        nc.gpsimd.tensor_copy(wf3[:, 0:D], w_sb[:])
        nc.gpsimd.tensor_copy(wf3[:, D:2 * D], wf3[:, 0:D])
        nc.gpsimd.tensor_copy(wf3[:, 2 * D:3 * D], wf3[:, 0:D])

        act_red = []        # deferred (l, xb) scalar-engine reductions
        xb_hold = sp.tile([P, LP * D], BF16)   # ACT_RED piece: no pool reuse
        pair_buf = None
        for i in range(NPC):
            rows = ROWS[i]
            if i == NPC - 1:
                # last piece entirely off the vector engine (the gap-free
                # critical path): gpsimd multiplies the landed fp32 tile
                # in place at arrival, scalar reduces the product
                x3g = xts[i][:, 0:rows * D].rearrange("p (l d) -> p l d", d=D)
                w3g = wf3[:, 0:rows * D].rearrange("p (l d) -> p l d", d=D)
                with tw(t_piece[i]):
                    nc.gpsimd.tensor_tensor(x3g, x3g, w3g, mult)
                for l in range(rows):
                    t = R0[i] + l
                    with tw(t_piece[i] + 0.8 + 0.3 * l):
                        nc.scalar.activation(
                            ascr2[:], xts[i][:, l * D:(l + 1) * D],
                            ACT.Copy, accum_out=xw[:, t:t + 1],
                        )
                continue
            # pieces 2..7 pair up in shared 12-row bf16 tiles so the
            # reduction stages run once per pair (per-instruction
            # overhead is ~25% of the stage cost); quads measured WORSE
            # (stage work piles up past the last cast, starving vector)
            in_pair = 2 <= i <= 7
            gsz = 2
            g0 = i - (i % 2)
            if in_pair and i == g0:
                pair_buf = b2.tile(
                    [P, gsz * LP * D], BF16, tag="x12", name="xbg"
                )
            if i == ACT_RED:
                xb, off = xb_hold, 0
            elif in_pair:
                xb, off = pair_buf, (i - g0) * LP * D
            else:
                xb = bp.tile([P, LP * D], BF16, tag="xb", name="xbs")
                off = 0
            x3 = xb[:, off:off + rows * D].rearrange("p (l d) -> p l d", d=D)
            w3 = wb[:, 0:rows * D].rearrange("p (l d) -> p l d", d=D)
            if i in (1, 2, 3):
                # vector casts these itself: it is otherwise idle waiting
                # for the scalar cast chain here (arrival-gated gaps), and
                # this relieves the scalar engine for the later casts
                with tw(t_piece[i] + 0.1):
                    nc.vector.tensor_copy(
                        xb[:, off:off + rows * D], xts[i][:, 0:rows * D]
                    )
            else:
                with tw(t_piece[i]):
                    nc.scalar.activation(
                        xb[:, off:off + rows * D], xts[i][:, 0:rows * D],
                        ACT.Copy,
                    )
            with tw(t_piece[i] + 0.25 * rows):
                nc.vector.tensor_tensor(x3, x3, w3, mult)
            if i == ACT_RED:
                act_red = [(R0[i] + l, xb) for l in range(rows)]
                continue
            if in_pair and i == g0 + gsz - 1:
                # staged reduce over the whole group at once
                p3 = pair_buf[:].rearrange("p (l d) -> p l d", d=D)
                with tw(t_piece[i] + 0.25 * rows + 0.9):
                    nc.vector.tensor_tensor(
                        p3[:, :, 0:128], p3[:, :, 0:128], p3[:, :, 128:256], add
                    )
                    nc.vector.tensor_tensor(
                        p3[:, :, 0:64], p3[:, :, 0:64], p3[:, :, 64:128], add
                    )
                    nc.vector.tensor_reduce(
                        xw[:, R0[g0]:R0[g0] + gsz * LP], p3[:, :, 0:64],
                        axis=AX, op=add,
                    )
            elif i == NPC - 1:
                # last piece's reductions run on the scalar engine, which
                # is idle by now — vector is the gap-free critical path
                # from ~19us to the end
                for l in range(rows):
                    t = R0[i] + l
                    with tw(t_piece[i] + 0.7 + 0.3 * l):
                        nc.scalar.activation(
                            ascr[:], xb[:, off + l * D:off + (l + 1) * D],
                            ACT.Copy, accum_out=xw[:, t:t + 1],
                        )
            elif not in_pair:
                with tw(t_piece[i] + 0.25 * rows + 0.9):
                    nc.vector.tensor_reduce(
                        xw[:, R0[i]:R0[i] + rows], x3, axis=AX, op=add
                    )
            # one deferred scalar-engine reduction per later piece: fills
            # the scalar engine's slack between casts
            if act_red and i >= 2:
                t, axb = act_red.pop(0)
                with tw(t_piece[i] + 1.8):
                    nc.scalar.activation(
                        ascr[:], axb[:, (t - R0[ACT_RED]) * D:
                                      (t - R0[ACT_RED] + 1) * D],
                        ACT.Copy, accum_out=xw[:, t:t + 1],
                    )

        # any leftovers (fewer later pieces than ACT_RED rows)
        for t, axb in act_red:
            with tw(t_piece[-1] + 1.0):
                nc.scalar.activation(
                    ascr[:], axb[:, (t - R0[ACT_RED]) * D:
                                  (t - R0[ACT_RED] + 1) * D],
                    ACT.Copy, accum_out=xw[:, t:t + 1],
                )

        TT = t_piece[-1] + 1.6
        # fold the valid-k mask into xw (masked -> -73, exp -> ~2e-32:
        # negligible in every suffix sum yet keeps ln() finite, so no
        # EPS seed is needed); wm is 0 there so loss terms are unaffected
        with tw(TT):
            nc.vector.tensor_add(xw[:], xw[:], mb_sb)
        # masked exponentials and chunk totals in one activation
        em = sp.tile([P, L], F32)
        tot = sp.tile([P, 1], BF16)
        with tw(TT + 0.1):
            nc.scalar.activation(em[:], xw[:], ACT.Exp, accum_out=tot[:])
        # sum(wm * xw) runs on vector in parallel with the exp: only the
        # ln-dependent half of the loss sum stays on the critical path
        diff = sp.tile([P, L], F32)
        with tw(TT + 0.1):
            nc.vector.scalar_tensor_tensor(
                out=diff[:], in0=xw[:], scalar=1.0, in1=wm_sb,
                op0=bypass, op1=mult, accum_out=res[:, 2:3],
            )
        # cross-chunk exclusive suffix totals via 128x128 bf16 matmul
        aps = pp.tile([P, 1], F32, tag="mm")
        with tw(TT + 0.2):
            nc.tensor.matmul(aps[:], umb[:], tot[:], start=True, stop=True)

        # within-chunk suffix sums run concurrently with the matmul
        # (seed 0); the later-chunk total is broadcast-added from PSUM
        ss = sp.tile([P, L], F32)
        with tw(TT + 0.25):
            nc.vector.tensor_tensor_scan(
                ss[:][:, ::-1], em[:][:, ::-1], em[:][:, ::-1],
                initial=0.0, op0=add, op1=bypass,
            )
        with tw(TT + 0.4):
            nc.vector.tensor_scalar_add(ss[:], ss[:], aps[:])
        lt = sp.tile([P, L], F32)
        with tw(TT + 0.5):
            nc.scalar.activation(lt[:], ss[:], ACT.Ln)

        # sum(wm * ln(suffix)); host computes num = col0 - col2
        with tw(TT + 0.6):
            nc.vector.scalar_tensor_tensor(
                out=diff[:], in0=lt[:], scalar=1.0, in1=wm_sb,
                op0=bypass, op1=mult, accum_out=res[:, 0:1],
            )
        with tw(TT + 0.8):
            nc.scalar.dma_start(out, res[:])

    nc.compile()
    return nc


def _host_consts():
    w_idx = np.arange(P)
    um = (
        (w_idx[:, None] // CH == w_idx[None, :] // CH)
        & (w_idx[:, None] % CH > w_idx[None, :] % CH)
    ).astype(np.float32)
    cm = np.ones((P, L), np.float32)
    cm[w_idx % CH == 0, 0] = 0.0
    return um, cm


def _host_blobs(mask, w_fc):
    """wt fp32, per-core mask blobs fp32, um bf16."""
    um, cm = _host_consts()
    wt = np.ascontiguousarray(
        np.broadcast_to(w_fc[HID:], (P, D)), np.float32
    )
    c2 = np.ascontiguousarray(um.astype(ml_dtypes.bfloat16))
    mfs = mask.astype(np.float32).reshape(NCORES, P, L)
    cbs = []
    for c in range(NCORES):
        mf = mfs[c]
        wm = mf * cm
        mb = (1.0 - mf) * MASK_NEG
        cbs.append(np.ascontiguousarray(
            np.concatenate([mf, wm, mb], axis=1), np.float32
        ))
    return wt, cbs, c2


def kernel(**inputs) -> np.ndarray:
    enc = np.ascontiguousarray(np.asarray(inputs["encoder_output"], np.float32))
    mask = np.ascontiguousarray(np.asarray(inputs["mask"], np.int32))
    w_fc = np.asarray(inputs["w_fc"], np.float32)

    if "nc" not in _cache:
        _cache["nc"] = _build_nc()
    nc = _cache["nc"]

    wt, cbs, c2 = _host_blobs(mask, w_fc)
    in_maps = [
        {"x": enc[c * BS:(c + 1) * BS], "wt": wt, "cb": cbs[c], "c2": c2}
        for c in range(NCORES)
    ]
    res = bass_utils.run_bass_kernel_spmd(
        nc, in_maps, core_ids=list(range(NCORES))
    )
    o = np.stack([r["out"] for r in res.results]).astype(np.float64)
    num = o[:, :, 0].sum() - o[:, :, 2].sum()
    den = o[:, :, 1].sum() - B
    return np.asarray(num / den, dtype=np.float32)


# revision 56
# speedup vs baseline: 1.0613x; 1.0331x over previous
"""Trainium2 Bass kernel for nn_DLI_loss_full.

Key algebraic fact: logits[b,j,k] = hw[b,j] + xw[b,k] and the loss is
sum(lse - tgt) over valid groups, so the hw[b,j] term (the whole LSTM
path) cancels exactly:

    per_group[b,j] = log(sum_{k=j+1}^{len_b-1} exp(xw[b,k])) - xw[b,j+1]
    loss = sum(per_group) / sum_b(len_b - 1)

with xw = encoder_output @ w_fc[HID:].  The kernel streams
encoder_output once (memory-bound, ~351 B/ns with all 16 DMA engines
saturated) on the sync HWDGE queue — the sync sequencer has nothing
else to do, so descriptor generation never delays compute issue — and
computes the per-timestep dot products as a two-engine pipeline with
no cross-engine back-edges (so the compile-time schedule cannot
head-of-line-block an engine):

  - scalar (ACT) casts each landed fp32 piece to bf16, and one piece's
    256-wide reductions run there via Copy+accum_out in the slack
    between casts,
  - vector (DVE) multiplies in the 2x 16-bit perf mode and reduces via
    two bf16 halving adds (2x mode) plus a short 1x reduce.

Pieces are sized 3+6*7+3 rows: a small first piece starts compute
~2.5us earlier and a small last piece keeps the post-stream tail
short.  tile_wait_until hints align the scheduler's per-engine
instruction order with the real DMA landing cadence.
Small constants (masks, um) ride the gpsimd SWDGE queue; they are only
needed in the tail.  A manual LoadActFuncSet of
natural_log_exp_and_others at the top of the scalar stream serves Exp
and Ln with a single hidden table load.  The valid-k mask folds into
xw as an additive -60000 bias so a single Exp+accum_out produces both
the masked exponentials and the chunk totals; suffix log-sum-exps come
from one per-chunk scan seeded through a 128x128 bf16 matmul.
"""

from contextlib import ExitStack

import ml_dtypes
import numpy as np

import concourse.bacc as bacc
import concourse.mybir as mybir
import concourse.tile as tile
from concourse import bass_utils

B, T, D, HID = 128, 384, 256, 256
NCORES = 8
BS = B // NCORES            # 16 batches per core
CH = 8                      # chunks per sequence
L = T // CH                 # 48 timesteps per chunk
P = BS * CH                 # 128 partitions
NP = 8                      # DMA/compute pieces along the free axis
LP = L // NP                # 6 timesteps per piece
F32 = mybir.dt.float32
BF16 = mybir.dt.bfloat16
EPS = 1e-30                 # keeps ln() finite on fully-masked tails
MASK_NEG = -73.0            # exp() ~ 2e-32: negligible yet nonzero, so
                            # suffix sums never hit ln(0) and no EPS
                            # seed is needed
ATL_LN_EXP = 6              # act_info.json index of natural_log_exp_and_others

_cache = {}


def _build_nc():
    nc = bacc.Bacc(
        "TRN2", target_bir_lowering=False, debug=False, num_devices=NCORES
    )
    x = nc.dram_tensor("x", [BS, T, D], F32, kind="ExternalInput").ap()
    wt = nc.dram_tensor("wt", [P, D], F32, kind="ExternalInput").ap()
    cb = nc.dram_tensor("cb", [P, 3 * L], F32, kind="ExternalInput").ap()
    c2 = nc.dram_tensor("c2", [P, P], BF16, kind="ExternalInput").ap()
    out = nc.dram_tensor("out", [P, 3], F32, kind="ExternalOutput").ap()

    add = mybir.AluOpType.add
    mult = mybir.AluOpType.mult
    bypass = mybir.AluOpType.bypass
    AX = mybir.AxisListType.X
    ACT = mybir.ActivationFunctionType

    with tile.TileContext(nc) as tc, ExitStack() as ctx, \
            nc.allow_low_precision(reason="bf16 2x-mode dot products; loss tolerance is 2e-2"):
        sp = ctx.enter_context(tc.tile_pool(name="small", bufs=1))
        xp = ctx.enter_context(tc.tile_pool(name="xp", bufs=10))
        bp = ctx.enter_context(tc.tile_pool(name="bp", bufs=4))
        b2 = ctx.enter_context(tc.tile_pool(name="b2", bufs=2))
        pp = ctx.enter_context(tc.tile_pool(name="psum", bufs=1, space="PSUM"))

        # piece row extents: small first piece so compute starts early,
        # small last piece so the post-stream tail is short
        ROWS = (3, 6, 6, 6, 6, 6, 6, 6, 3)
        R0 = [sum(ROWS[:i]) for i in range(len(ROWS))]
        NPC = len(ROWS)
        ACT_RED = 1         # this piece's reductions run on scalar Copy+accum
        GP_A = GP_V = -1    # gpsimd tail offload: measured slower, disabled

        # expected piece-i landing time in the scheduler's simulated
        # clock (serial DMA queue model): used only to pin instruction
        # order per engine, no hardware waits are emitted
        def tw(us):
            return tc.tile_wait_until(us * 1e-3)

        t_piece = []
        t = 0.8
        for r in ROWS:
            t += 2.37 * r / 6.0
            t_piece.append(t)

        # one table load serves every Exp/Ln below; runs at stream head
        # while w / the first x piece are still in flight
        nc.scalar.add_instruction(
            mybir.InstLoadActFuncSet(
                name="manual_atl", act_func_set_id=ATL_LN_EXP, ins=[], outs=[]
            )
        )

        # w first, then the x pieces, all on the sync HWDGE queue: 128
        # descriptors of 6KB per piece keep all 16 DMA engines saturated,
        # and the sync sequencer has nothing else to do, so descriptor
        # generation never blocks compute issue (on the scalar queue the
        # piece descriptor-gens delayed every activation by ~7us)
        w_sb = sp.tile([P, D], F32)
        nc.sync.dma_start(w_sb[:], wt)
        x_p = x.rearrange("b (c l) d -> (b c) (l d)", c=CH)
        xts = []
        for i in range(NPC):
            xt = xp.tile([P, LP * D], F32, tag="x")
            nc.sync.dma_start(
                xt[:, 0:ROWS[i] * D],
                x_p[:, R0[i] * D:(R0[i] + ROWS[i]) * D],
            )
            xts.append(xt)

        # small tail-only constants ride the gpsimd SWDGE queue
        c_sb = sp.tile([P, 3 * L], F32)
        nc.gpsimd.dma_start(c_sb[:], cb)
        mf_sb = c_sb[:, 0:L]
        wm_sb = c_sb[:, L:2 * L]
        mb_sb = c_sb[:, 2 * L:3 * L]
        umb = sp.tile([P, P], BF16)
        nc.gpsimd.dma_start(umb[:], c2)

        # bf16 weight replica, built while the first piece is in flight
        wtb = sp.tile([P, D], BF16)
        nc.scalar.activation(wtb[:], w_sb[:], ACT.Copy)
        wb = sp.tile([P, LP * D], BF16)
        nc.vector.tensor_copy(wb[:, 0:D], wtb[:])
        nc.vector.tensor_copy(wb[:, D:2 * D], wb[:, 0:D])
        nc.vector.tensor_copy(wb[:, 2 * D:4 * D], wb[:, 0:2 * D])
        nc.vector.tensor_copy(wb[:, 4 * D:6 * D], wb[:, 2 * D:4 * D])
        w3b = wb[:].rearrange("p (l d) -> p l d", d=D)

        res = sp.tile([P, 3], F32)
        # valid-group count is independent of x: do it up front
        nc.vector.tensor_reduce(res[:, 1:2], mf_sb, axis=AX, op=add)

        # xw[p, t] = sum_d x[p, t, d] * w[d]
        xw = sp.tile([P, L], F32)
        ascr = sp.tile([P, D], BF16)

        # fp32 weight replica (3 rows) for gpsimd's last-piece multiply
        wf3 = sp.tile([P, 3 * D], F32)
        ascr2 = sp.tile([P, D], F32)
        nc.gpsimd.tensor_copy(wf3[:, 0:D], w_sb[:])
        nc.gpsimd.tensor_copy(wf3[:, D:2 * D], wf3[:, 0:D])
        nc.gpsimd.tensor_copy(wf3[:, 2 * D:3 * D], wf3[:, 0:D])

        act_red = []        # deferred (l, xb) scalar-engine reductions
        xb_hold = sp.tile([P, LP * D], BF16)   # ACT_RED piece: no pool reuse
        pair_buf = None
        for i in range(NPC):
            rows = ROWS[i]
            if i == NPC - 1:
                # last piece entirely off the vector engine (the gap-free
                # critical path): gpsimd multiplies the landed fp32 tile
                # in place at arrival, scalar reduces the product
                x3g = xts[i][:, 0:rows * D].rearrange("p (l d) -> p l d", d=D)
                w3g = wf3[:, 0:rows * D].rearrange("p (l d) -> p l d", d=D)
                with tw(t_piece[i]):
                    nc.gpsimd.tensor_tensor(x3g, x3g, w3g, mult)
                for l in range(rows):
                    t = R0[i] + l
                    with tw(t_piece[i] + 0.8 + 0.3 * l):
                        nc.scalar.activation(
                            ascr2[:], xts[i][:, l * D:(l + 1) * D],
                            ACT.Copy, accum_out=xw[:, t:t + 1],
                        )
                continue
            # pieces 2..7 pair up in shared 12-row bf16 tiles so the
            # reduction stages run once per pair (per-instruction
            # overhead is ~25% of the stage cost); quads measured WORSE
            # (stage work piles up past the last cast, starving vector)
            in_pair = 2 <= i <= 7
            gsz = 2
            g0 = i - (i % 2)
            if in_pair and i == g0:
                pair_buf = b2.tile(
                    [P, gsz * LP * D], BF16, tag="x12", name="xbg"
                )
            if i == ACT_RED:
                xb, off = xb_hold, 0
            elif in_pair:
                xb, off = pair_buf, (i - g0) * LP * D
            else:
                xb = bp.tile([P, LP * D], BF16, tag="xb", name="xbs")
                off = 0
            x3 = xb[:, off:off + rows * D].rearrange("p (l d) -> p l d", d=D)
            w3 = wb[:, 0:rows * D].rearrange("p (l d) -> p l d", d=D)
            if i in (1, 2, 3):
                # vector casts these itself: it is otherwise idle waiting
                # for the scalar cast chain here (arrival-gated gaps), and
                # this relieves the scalar engine for the later casts
                with tw(t_piece[i] + 0.1):
                    nc.vector.tensor_copy(
                        xb[:, off:off + rows * D], xts[i][:, 0:rows * D]
                    )
            else:
                with tw(t_piece[i]):
                    nc.scalar.activation(
                        xb[:, off:off + rows * D], xts[i][:, 0:rows * D],
                        ACT.Copy,
                    )
            with tw(t_piece[i] + 0.25 * rows):
                nc.vector.tensor_tensor(x3, x3, w3, mult)
            if i == ACT_RED:
                act_red = [(R0[i] + l, xb) for l in range(rows)]
                continue
            if in_pair and i == g0 + gsz - 1:
                # staged reduce over the whole group at once
                p3 = pair_buf[:].rearrange("p (l d) -> p l d", d=D)
                with tw(t_piece[i] + 0.25 * rows + 0.9):
                    nc.vector.tensor_tensor(
                        p3[:, :, 0:128], p3[:, :, 0:128], p3[:, :, 128:256], add
                    )
                    nc.vector.tensor_tensor(
                        p3[:, :, 0:64], p3[:, :, 0:64], p3[:, :, 64:128], add
                    )
                    nc.vector.tensor_reduce(
                        xw[:, R0[g0]:R0[g0] + gsz * LP], p3[:, :, 0:64],
                        axis=AX, op=add,
                    )
            elif i == NPC - 1:
                # last piece's reductions run on the scalar engine, which
                # is idle by now — vector is the gap-free critical path
                # from ~19us to the end
                for l in range(rows):
                    t = R0[i] + l
                    with tw(t_piece[i] + 0.7 + 0.3 * l):
                        nc.scalar.activation(
                            ascr[:], xb[:, off + l * D:off + (l + 1) * D],
                            ACT.Copy, accum_out=xw[:, t:t + 1],
                        )
            elif not in_pair:
                with tw(t_piece[i] + 0.25 * rows + 0.9):
                    nc.vector.tensor_reduce(
                        xw[:, R0[i]:R0[i] + rows], x3, axis=AX, op=add
                    )
            # one deferred scalar-engine reduction per later piece: fills
            # the scalar engine's slack between casts
            if act_red and i >= 2:
                t, axb = act_red.pop(0)
                with tw(t_piece[i] + 1.8):
                    nc.scalar.activation(
                        ascr[:], axb[:, (t - R0[ACT_RED]) * D:
                                      (t - R0[ACT_RED] + 1) * D],
                        ACT.Copy, accum_out=xw[:, t:t + 1],
                    )

        # any leftovers (fewer later pieces than ACT_RED rows)
        for t, axb in act_red:
            with tw(t_piece[-1] + 1.0):
                nc.scalar.activation(
                    ascr[:], axb[:, (t - R0[ACT_RED]) * D:
                                  (t - R0[ACT_RED] + 1) * D],
                    ACT.Copy, accum_out=xw[:, t:t + 1],
                )

        TT = t_piece[-1] + 1.6
        # fold the valid-k mask into xw (masked -> -73, exp -> ~2e-32:
        # negligible in every suffix sum yet keeps ln() finite, so no
        # EPS seed is needed); wm is 0 there so loss terms are unaffected
        with tw(TT):
            nc.vector.tensor_add(xw[:], xw[:], mb_sb)
        # masked exponentials and chunk totals in one activation
        em = sp.tile([P, L], F32)
        tot = sp.tile([P, 1], BF16)
        with tw(TT + 0.1):
            nc.scalar.activation(em[:], xw[:], ACT.Exp, accum_out=tot[:])
        # sum(wm * xw) runs on vector in parallel with the exp: only the
        # ln-dependent half of the loss sum stays on the critical path
        diff = sp.tile([P, L], F32)
        with tw(TT + 0.1):
            nc.vector.scalar_tensor_tensor(
                out=diff[:], in0=xw[:], scalar=1.0, in1=wm_sb,
                op0=bypass, op1=mult, accum_out=res[:, 2:3],
            )
        # cross-chunk exclusive suffix totals via 128x128 bf16 matmul
        aps = pp.tile([P, 1], F32, tag="mm")
        with tw(TT + 0.2):
            nc.tensor.matmul(aps[:], umb[:], tot[:], start=True, stop=True)

        # within-chunk suffix sums, seeded with the later-chunk total
        # (read straight from PSUM)
        ss = sp.tile([P, L], F32)
        with tw(TT + 0.4):
            nc.vector.tensor_tensor_scan(
                ss[:][:, ::-1], em[:][:, ::-1], em[:][:, ::-1],
                initial=aps[:], op0=add, op1=bypass,
            )
        lt = sp.tile([P, L], F32)
        with tw(TT + 0.5):
            nc.scalar.activation(lt[:], ss[:], ACT.Ln)

        # sum(wm * ln(suffix)); host computes num = col0 - col2
        with tw(TT + 0.6):
            nc.vector.scalar_tensor_tensor(
                out=diff[:], in0=lt[:], scalar=1.0, in1=wm_sb,
                op0=bypass, op1=mult, accum_out=res[:, 0:1],
            )
        with tw(TT + 0.8):
            nc.scalar.dma_start(out, res[:])

    nc.compile()
    return nc


def _host_consts():
    w_idx = np.arange(P)
    um = (
        (w_idx[:, None] // CH == w_idx[None, :] // CH)
        & (w_idx[:, None] % CH > w_idx[None, :] % CH)
    ).astype(np.float32)
    cm = np.ones((P, L), np.float32)
    cm[w_idx % CH == 0, 0] = 0.0
    return um, cm


def _host_blobs(mask, w_fc):
    """wt fp32, per-core mask blobs fp32, um bf16."""
    um, cm = _host_consts()
    wt = np.ascontiguousarray(
        np.broadcast_to(w_fc[HID:], (P, D)), np.float32
    )
    c2 = np.ascontiguousarray(um.astype(ml_dtypes.bfloat16))
    mfs = mask.astype(np.float32).reshape(NCORES, P, L)
    cbs = []
    for c in range(NCORES):
        mf = mfs[c]
        wm = mf * cm
        mb = (1.0 - mf) * MASK_NEG
        cbs.append(np.ascontiguousarray(
            np.concatenate([mf, wm, mb], axis=1), np.float32
        ))
    return wt, cbs, c2


def kernel(**inputs) -> np.ndarray:
    enc = np.ascontiguousarray(np.asarray(inputs["encoder_output"], np.float32))
    mask = np.ascontiguousarray(np.asarray(inputs["mask"], np.int32))
    w_fc = np.asarray(inputs["w_fc"], np.float32)

    if "nc" not in _cache:
        _cache["nc"] = _build_nc()
    nc = _cache["nc"]

    wt, cbs, c2 = _host_blobs(mask, w_fc)
    in_maps = [
        {"x": enc[c * BS:(c + 1) * BS], "wt": wt, "cb": cbs[c], "c2": c2}
        for c in range(NCORES)
    ]
    res = bass_utils.run_bass_kernel_spmd(
        nc, in_maps, core_ids=list(range(NCORES))
    )
    o = np.stack([r["out"] for r in res.results]).astype(np.float64)
    num = o[:, :, 0].sum() - o[:, :, 2].sum()
    den = o[:, :, 1].sum() - B
    return np.asarray(num / den, dtype=np.float32)
